# revision 48
# baseline (speedup 1.0000x reference)
"""Multi-head attention Bass/Tile kernel for TRN2, 8-core SPMD.

Sharding: core c handles batch b = c//2 and head-group g = c%2 (6 of 12 heads).
Each core computes its 6 heads end-to-end plus a partial output projection
(over its 384 of 768 ctx dims); the host sums the two partials per batch.

Design: all matmuls bf16 (PSUM accumulation stays f32). The ScalarE exp
stream (192 x [128,1024] ~ 208us) is the hard floor; everything else hides
under it on a dense PE queue:
  - inputs packed host-side as [128, chunk, cols] so each partition's data
    is one contiguous DRAM range (few, large DMA descriptors)
  - PE warmup matmuls during the DMA window (p-state ramps 0.65->2.4 GHz)
  - lead-in computes only m0 qT/kT; v tiles are produced just-in-time and
    m1/m2 q/k chains run as background units inside the attention loop
  - score pairs K=64 run row-tiled (concurrent halves), one exp instruction
    covers both heads of a pair, ctx accumulates pend-by-1
  - per-pair softmax normalization: one [2,512] reciprocal, gpsimd
    partition-broadcast, DVE multiplies
  - out-proj fused per s-block into the next block's background slots
"""

from collections import deque
from contextlib import ExitStack

import numpy as np
import ml_dtypes

import concourse.bass as bass
import concourse.tile as tile
from concourse import bacc, mybir
from concourse._compat import with_exitstack
from concourse.dve_ops import RECIPROCAL_APPROX_FAST, RECIP_APPROX_FAST_CONSTS

F32 = mybir.dt.float32
BF16 = mybir.dt.bfloat16
AF = mybir.ActivationFunctionType

B, E, S, H, D = 4, 768, 2048, 12, 64
NH = 6          # heads per core
HD = NH * D     # 384 head-dims per core
NE = E // 128   # 6 e-chunks
NM = HD // 128  # 3 m-chunks (2 heads each)
NT = S // 128   # 16 t-tiles
SBW = 512       # s-block width
NS = S // SBW   # 4 s-blocks
VW = 96         # ctx stationary width: col 0 = ones (denominator), 32:96 = v
WARMUP = 24


@with_exitstack
def mha_tile(ctx: ExitStack, tc, hs, wq, wk, wv, bq, bk, bv, woT, bo2, outT):
    nc = tc.nc

    persist = ctx.enter_context(tc.tile_pool(name="persist", bufs=1))

    # --- persistent SBUF tiles (packed layouts match the DRAM packing) ---
    hs_sb = persist.tile([128, NE, S], BF16, name="hs")
    wq_sb = persist.tile([128, NE, HD], BF16, name="wq")
    wk_sb = persist.tile([128, NE, HD], BF16, name="wk")
    wv_sb = persist.tile([128, NE, HD], BF16, name="wv")
    woT_sb = persist.tile([128, NM, E], BF16, name="wo")
    qT_sb = [persist.tile([128, S], BF16, name=f"qT{m}") for m in range(NM)]
    kT_sb = [persist.tile([128, S], BF16, name=f"kT{m}") for m in range(NM)]
    ctxT_sb = [persist.tile([128, S], BF16, name=f"ctxT{m}") for m in range(NM)]
    v_aug = [persist.tile([128, NH, VW], BF16, name=f"vaug{t}") for t in range(NT)]


    bq_sb = persist.tile([128, NM], F32, name="bq")
    bk_sb = persist.tile([128, NM], F32, name="bk")
    bv_bc = persist.tile([128, HD], F32, name="bv")
    bo_sb = persist.tile([128, NE], F32, name="bo")

    # --- DMA: large contiguous-per-partition transfers, split in halves so
    # two queues run in parallel per tensor ---
    nc.sync.dma_start(hs_sb[:], hs)
    nc.sync.dma_start(wq_sb[:], wq)
    nc.sync.dma_start(wk_sb[:], wk)
    nc.sync.dma_start(wv_sb[:], wv)
    nc.sync.dma_start(woT_sb[:], woT)
    nc.sync.dma_start(bq_sb[:], bq.rearrange("(m p) -> p m", p=128))
    nc.sync.dma_start(bk_sb[:], bk.rearrange("(m p) -> p m", p=128))
    nc.sync.dma_start(
        bv_bc[:], bass.AP(tensor=bv.tensor, offset=bv.offset, ap=[[0, 128], [1, HD]])
    )
    nc.sync.dma_start(bo_sb[:], bo2.rearrange("(m p) -> p m", p=128))

    # v_aug init on gpsimd (vector stays free): zeros, then ones in col 0
    for t in range(NT):
        nc.gpsimd.memset(v_aug[t][:].rearrange("p h d -> p (h d)"), 0.0)
        nc.gpsimd.memset(v_aug[t][:, :, 0:1], 1.0)

    # --- PSUM pools (8 banks: 4 sc + 2 ctx + 2 misc) ---
    pssc = ctx.enter_context(tc.tile_pool(name="pssc", bufs=2, space="PSUM"))
    psctx = ctx.enter_context(tc.tile_pool(name="psctx", bufs=1, space="PSUM"))
    psmisc = ctx.enter_context(tc.tile_pool(name="psmisc", bufs=2, space="PSUM"))

    # SBUF working pools
    expp = ctx.enter_context(tc.tile_pool(name="expp", bufs=5))
    cup = ctx.enter_context(tc.tile_pool(name="cup", bufs=6))
    bcp = ctx.enter_context(tc.tile_pool(name="bcp", bufs=8))
    outp = ctx.enter_context(tc.tile_pool(name="outp", bufs=2))

    # --- PE warmup during the DMA window: ramps the tensor engine to full
    # p-state before real work. Outputs are consumed to keep walrus honest.
    scratch = persist.tile([128, SBW], BF16, name="scratch")
    scon = persist.tile([128, SBW], F32, name="scon")
    nc.vector.memset(scratch[:], 0.0)
    for i in range(WARMUP):
        wp = psmisc.tile([128, SBW], F32, tag="misc")
        nc.tensor.matmul(wp[:], scratch[:, 0:128], scratch[:], start=True, stop=True)
        if i % 8 == 7:
            nc.vector.tensor_copy(scon[:], wp[:])

    # --- work units ---
    def q_unit(m, s):
        msl = slice(128 * m, 128 * (m + 1))
        ssl = slice(SBW * s, SBW * (s + 1))
        qp = psmisc.tile([128, SBW], F32, tag="misc")
        for e in range(NE):
            nc.tensor.matmul(
                qp[:], wq_sb[:, e, msl], hs_sb[:, e, ssl],
                start=(e == 0), stop=(e == NE - 1),
            )
        nc.vector.tensor_scalar_add(
            out=qT_sb[m][:, ssl], in0=qp[:], scalar1=bq_sb[:, m : m + 1]
        )

    def k_unit(m, s):
        msl = slice(128 * m, 128 * (m + 1))
        ssl = slice(SBW * s, SBW * (s + 1))
        kp = psmisc.tile([128, SBW], F32, tag="misc")
        for e in range(NE):
            nc.tensor.matmul(
                kp[:], wk_sb[:, e, msl], hs_sb[:, e, ssl],
                start=(e == 0), stop=(e == NE - 1),
            )
        nc.vector.tensor_scalar_add(
            out=kT_sb[m][:, ssl], in0=kp[:], scalar1=bk_sb[:, m : m + 1]
        )

    def v_unit(t):
        tsl = slice(128 * t, 128 * (t + 1))
        vp = psmisc.tile([128, SBW], F32, tag="misc")
        for e in range(NE):
            nc.tensor.matmul(
                vp[:, 0:HD], hs_sb[:, e, tsl], wv_sb[:, e, :],
                start=(e == 0), stop=(e == NE - 1),
            )
        nc.vector.tensor_add(
            out=v_aug[t][:, :, 32 : 32 + D],
            in0=vp[:, 0:HD].rearrange("p (h d) -> p h d", h=NH),
            in1=bv_bc[:].rearrange("p (h d) -> p h d", h=NH),
        )

    def out_unit(s, et):
        esl = slice(128 * et, 128 * (et + 1))
        ssl = slice(SBW * s, SBW * (s + 1))
        op = psmisc.tile([128, SBW], F32, tag="misc")
        for f in range(NM):
            nc.tensor.matmul(
                op[:], woT_sb[:, f, esl], ctxT_sb[f][:, ssl],
                start=(f == 0), stop=(f == NM - 1),
            )
        ob = outp.tile([128, SBW], F32, tag="ob")
        nc.vector.tensor_scalar_add(
            out=ob[:], in0=op[:], scalar1=bo_sb[:, et : et + 1]
        )
        nc.sync.dma_start(outT[esl, ssl], ob[:])

    # --- lead-in: just the two chains the first score tiles need; scores
    # for t-tiles 4k.. wait on k_unit(0, k) which streams in as background
    k_unit(0, 0)
    q_unit(0, 0)

    bg = deque()          # v tiles just-in-time: v[t] issued at step t
    for t in range(NT):
        bg.append(lambda t=t: v_unit(t))
    bg2 = deque()         # remaining m0 chains, then m1/m2, popped per step
    for s in range(1, NS):
        bg2.append(lambda s=s: k_unit(0, s))
    for s in range(1, NS):
        bg2.append(lambda s=s: q_unit(0, s))
    for m in (1, 2):
        for s in range(NS):
            bg2.append(lambda m=m, s=s: k_unit(m, s))
            bg2.append(lambda m=m, s=s: q_unit(m, s))
    bgout = deque()       # fused out-proj units

    # --- phase 2 ---
    pending_norm = [None]  # deferred normalize-scale of the previous pair

    def norm_scale(p, ssl, cu2):
        """Z -> partition-broadcast (from cu row 0) -> fast reciprocal on the
        broadcast (partition-parallel, same cost) -> scale ctx into ctxT.
        No DMA hops; deferred into the NEXT pair so it never gates PSUM."""
        c = RECIP_APPROX_FAST_CONSTS
        for a in range(2):
            bcz = bcp.tile([VW, SBW], F32, tag="bcs")
            nc.gpsimd.partition_broadcast(bcz[:], cu2[a][0:1, :])
            bci = bcp.tile([VW, SBW], F32, tag="bcs")
            nc.vector._custom_dve(
                RECIPROCAL_APPROX_FAST, out=bci[:], in0=bcz[:],
                s0=c["s0"], s1=c["s1"], imm2=c["imm2"],
            )
            for q in range(2):  # 32-partition chunks (alignment rules)
                nc.vector.tensor_mul(
                    out=ctxT_sb[p][D * a + 32 * q : D * a + 32 * (q + 1), ssl],
                    in0=cu2[a][32 + 32 * q : 64 + 32 * q, :],
                    in1=bci[32 + 32 * q : 64 + 32 * q, :],
                )

    for s in range(NS):
        ssl = slice(SBW * s, SBW * (s + 1))
        for p in range(NM):
            kTh = kT_sb[p]
            qTh = qT_sb[p]
            cpA = psctx.tile([128, SBW], F32, tag="ctxA")
            cpB = psctx.tile([128, SBW], F32, tag="ctxB")
            pend = deque()  # depth-2: ctx consumes exp from 2 steps ago,
            # so its semaphore wait is pre-satisfied (no PE stall)

            def ctx_mms(ex, t, stop):
                st = t == 0
                nc.tensor.matmul(
                    cpA[0:VW, :], v_aug[t][:, 2 * p, :], ex[:, 0:SBW],
                    start=st, stop=stop,
                )
                nc.tensor.matmul(
                    cpB[0:VW, :], v_aug[t][:, 2 * p + 1, :], ex[:, SBW : 2 * SBW],
                    start=st, stop=stop,
                )

            for t in range(NT):
                tsl = slice(128 * t, 128 * (t + 1))
                if len(pend) == 2:
                    ex2, t2 = pend.popleft()
                    ctx_mms(ex2, t2, stop=False)
                sc = pssc.tile([128, 2 * SBW], F32, tag="sc")
                nc.tensor.matmul(
                    sc[:, 0:SBW], kTh[0:D, tsl], qTh[0:D, ssl],
                    start=True, stop=True,
                )
                nc.tensor.matmul(
                    sc[:, SBW : 2 * SBW], kTh[D:128, tsl], qTh[D:128, ssl],
                    start=True, stop=True,
                )
                ex = expp.tile([128, 2 * SBW], BF16, tag="exp")
                nc.scalar.activation(ex[:], sc[:], AF.Exp)
                pend.append((ex, t))
                # deferred normalize-scale of the previous pair, early in
                # this pair (its DVE work now sits behind nothing critical)
                if t == 2 and pending_norm[0] is not None:
                    pending_norm[0]()
                    pending_norm[0] = None
                # background fill: v just-in-time, qk chains every other
                # step, out-proj only after the first few steps of p0 (its
                # ctxT inputs finish normalizing early in this pair)
                if bg:
                    bg.popleft()()
                if bg2 and t < 14:
                    bg2.popleft()()
                elif bgout and (p > 0 or t >= 4) and t < 14:
                    bgout.popleft()()
            while pend:
                ex2, t2 = pend.popleft()
                ctx_mms(ex2, t2, stop=(t2 == NT - 1))

            # per-pair normalize, part 1: evacuate ctx PSUM promptly (frees
            # the banks for the next pair) and collect the denominators
            cu2 = []
            for a, cp in ((0, cpA), (1, cpB)):
                cu = cup.tile([VW, SBW], F32, tag="cu")
                nc.vector.tensor_copy(cu[:], cp[0:VW, :])
                cu2.append(cu)
            if s == NS - 1 and p == NM - 1:
                norm_scale(p, ssl, cu2)  # last pair: inline, shortens tail
            else:
                pending_norm[0] = lambda p=p, ssl=ssl, cu2=cu2: norm_scale(p, ssl, cu2)
        for et in range(NE):
            bgout.append(lambda s=s, et=et: out_unit(s, et))

    # --- drain: final pair's normalize, then remaining out-proj ---
    if pending_norm[0] is not None:
        pending_norm[0]()
        pending_norm[0] = None
    while bg2:
        bg2.popleft()()
    while bgout:
        bgout.popleft()()


def build_nc():
    nc = bacc.Bacc("TRN2", target_bir_lowering=False, debug=False)
    hs = nc.dram_tensor("hs", [128, NE, S], BF16, kind="ExternalInput")
    wq = nc.dram_tensor("wq", [128, NE, HD], BF16, kind="ExternalInput")
    wk = nc.dram_tensor("wk", [128, NE, HD], BF16, kind="ExternalInput")
    wv = nc.dram_tensor("wv", [128, NE, HD], BF16, kind="ExternalInput")
    bq = nc.dram_tensor("bq", [HD], F32, kind="ExternalInput")
    bk = nc.dram_tensor("bk", [HD], F32, kind="ExternalInput")
    bv = nc.dram_tensor("bv", [HD], F32, kind="ExternalInput")
    woT = nc.dram_tensor("woT", [128, NM, E], BF16, kind="ExternalInput")
    bo2 = nc.dram_tensor("bo2", [E], F32, kind="ExternalInput")
    outT = nc.dram_tensor("outT", [E, S], F32, kind="ExternalOutput")

    with tile.TileContext(nc) as tc:
        mha_tile(
            tc,
            hs[:, :, :], wq[:, :, :], wk[:, :, :], wv[:, :, :],
            bq[:], bk[:], bv[:],
            woT[:, :, :], bo2[:], outT[:, :],
        )
    nc.compile()
    return nc


def _pack(x: np.ndarray, nchunk: int) -> np.ndarray:
    """[nchunk*128, cols] row-major -> [128, nchunk, cols] (partition-major)."""
    bf16 = ml_dtypes.bfloat16
    cols = x.shape[1]
    return np.ascontiguousarray(
        x.reshape(nchunk, 128, cols).transpose(1, 0, 2).astype(bf16)
    )


def make_core_inputs(inputs: dict) -> list[dict]:
    """Full inputs -> per-core input maps (core c: batch c//2, head-group c%2)."""
    hsf = np.ascontiguousarray(np.asarray(inputs["hidden_state"], dtype=np.float32))
    Wq = np.asarray(inputs["Wq"], dtype=np.float32)
    Wk = np.asarray(inputs["Wk"], dtype=np.float32)
    Wv = np.asarray(inputs["Wv"], dtype=np.float32)
    Wo = np.asarray(inputs["Wo"], dtype=np.float32)
    bq = np.asarray(inputs["bq"], dtype=np.float32)
    bk = np.asarray(inputs["bk"], dtype=np.float32)
    bv = np.asarray(inputs["bv"], dtype=np.float32)
    bo = np.asarray(inputs["bo"], dtype=np.float32)

    maps = []
    for c in range(8):
        b, g = c // 2, c % 2
        hsl = slice(NH * g, NH * (g + 1))
        fsl = slice(HD * g, HD * (g + 1))
        maps.append(
            {
                "hs": _pack(hsf[b], NE),
                "wq": _pack(Wq[hsl].transpose(1, 0, 2).reshape(E, HD), NE),
                "wk": _pack(Wk[hsl].transpose(1, 0, 2).reshape(E, HD), NE),
                "wv": _pack(Wv[hsl].transpose(1, 0, 2).reshape(E, HD), NE),
                "bq": np.ascontiguousarray(bq[hsl].reshape(HD)),
                "bk": np.ascontiguousarray(bk[hsl].reshape(HD)),
                "bv": np.ascontiguousarray(bv[hsl].reshape(HD)),
                "woT": _pack(np.ascontiguousarray(Wo[:, fsl].T), NM),
                "bo2": np.ascontiguousarray(bo / 2.0),
            }
        )
    return maps


def combine_outputs(core_outs: list) -> np.ndarray:
    """Per-core outT partials -> full [B, E, S] output."""
    return np.stack(
        [core_outs[2 * b]["outT"] + core_outs[2 * b + 1]["outT"] for b in range(B)]
    ).astype(np.float32)


from concourse.bass_utils import run_bass_kernel_spmd

N_CORES = 8
_NC_CACHE = None


def _get_nc():
    global _NC_CACHE
    if _NC_CACHE is None:
        _NC_CACHE = build_nc()
    return _NC_CACHE


def kernel(**inputs) -> np.ndarray:
    """Full-input entry point: shard across 8 cores, run, unshard."""
    maps = make_core_inputs(inputs)
    nc = _get_nc()
    res = run_bass_kernel_spmd(nc, maps, core_ids=list(range(N_CORES)))
    outs = res.results
    return np.stack(
        [outs[2 * b]["outT"] + outs[2 * b + 1]["outT"] for b in range(B)]
    ).astype(np.float32)


# revision 51
# speedup vs baseline: 1.0133x; 1.0133x over previous
"""Multi-head attention Bass/Tile kernel for TRN2, 8-core SPMD.

Sharding: core c handles batch b = c//2 and head-group g = c%2 (6 of 12 heads).
Each core computes its 6 heads end-to-end plus a partial output projection
(over its 384 of 768 ctx dims); the host sums the two partials per batch.

Design: all matmuls bf16 (PSUM accumulation stays f32). The ScalarE exp
stream (192 x [128,1024] ~ 208us) is the hard floor; everything else hides
under it on a dense PE queue:
  - inputs packed host-side as [128, chunk, cols] so each partition's data
    is one contiguous DRAM range (few, large DMA descriptors)
  - PE warmup matmuls during the DMA window (p-state ramps 0.65->2.4 GHz)
  - lead-in computes only m0 qT/kT; v tiles are produced just-in-time and
    m1/m2 q/k chains run as background units inside the attention loop
  - score pairs K=64 run row-tiled (concurrent halves), one exp instruction
    covers both heads of a pair, ctx accumulates pend-by-1
  - per-pair softmax normalization: one [2,512] reciprocal, gpsimd
    partition-broadcast, DVE multiplies
  - out-proj fused per s-block into the next block's background slots
"""

from collections import deque
from contextlib import ExitStack

import numpy as np
import ml_dtypes

import concourse.bass as bass
import concourse.tile as tile
from concourse import bacc, mybir
from concourse._compat import with_exitstack
from concourse.dve_ops import RECIPROCAL_APPROX_FAST, RECIP_APPROX_FAST_CONSTS

F32 = mybir.dt.float32
BF16 = mybir.dt.bfloat16
AF = mybir.ActivationFunctionType

B, E, S, H, D = 4, 768, 2048, 12, 64
NH = 6          # heads per core
HD = NH * D     # 384 head-dims per core
NE = E // 128   # 6 e-chunks
NM = HD // 128  # 3 m-chunks (2 heads each)
NT = S // 128   # 16 t-tiles
SBW = 512       # s-block width
NS = S // SBW   # 4 s-blocks
VW = 96         # ctx stationary width: col 0 = ones (denominator), 32:96 = v
WARMUP = 24


@with_exitstack
def mha_tile(ctx: ExitStack, tc, hs, wq, wk, wv, bq, bk, bv, woT, bo2, outT):
    nc = tc.nc

    persist = ctx.enter_context(tc.tile_pool(name="persist", bufs=1))

    # --- persistent SBUF tiles (packed layouts match the DRAM packing) ---
    hs_sb = persist.tile([128, NE, S], BF16, name="hs")
    wq_sb = persist.tile([128, NE, HD], BF16, name="wq")
    wk_sb = persist.tile([128, NE, HD], BF16, name="wk")
    wv_sb = persist.tile([128, NE, HD], BF16, name="wv")
    woT_sb = persist.tile([128, NM, E], BF16, name="wo")
    qT_sb = [persist.tile([128, S], BF16, name=f"qT{m}") for m in range(NM)]
    kT_sb = [persist.tile([128, S], BF16, name=f"kT{m}") for m in range(NM)]
    ctxT_sb = [persist.tile([128, S], BF16, name=f"ctxT{m}") for m in range(NM)]
    v_aug = [persist.tile([128, NH, VW], BF16, name=f"vaug{t}") for t in range(NT)]


    bq_sb = persist.tile([128, NM], F32, name="bq")
    bk_sb = persist.tile([128, NM], F32, name="bk")
    bv_bc = persist.tile([128, HD], F32, name="bv")
    bo_sb = persist.tile([128, NE], F32, name="bo")

    # --- DMA: large contiguous-per-partition transfers, split in halves so
    # two queues run in parallel per tensor ---
    nc.sync.dma_start(hs_sb[:], hs)
    nc.sync.dma_start(wq_sb[:], wq)
    nc.sync.dma_start(wk_sb[:], wk)
    nc.sync.dma_start(wv_sb[:], wv)
    nc.sync.dma_start(woT_sb[:], woT)
    nc.sync.dma_start(bq_sb[:], bq.rearrange("(m p) -> p m", p=128))
    nc.sync.dma_start(bk_sb[:], bk.rearrange("(m p) -> p m", p=128))
    nc.sync.dma_start(
        bv_bc[:], bass.AP(tensor=bv.tensor, offset=bv.offset, ap=[[0, 128], [1, HD]])
    )
    nc.sync.dma_start(bo_sb[:], bo2.rearrange("(m p) -> p m", p=128))

    # v_aug init on gpsimd (vector stays free): zeros, then ones in col 0
    for t in range(NT):
        nc.gpsimd.memset(v_aug[t][:].rearrange("p h d -> p (h d)"), 0.0)
        nc.gpsimd.memset(v_aug[t][:, :, 0:1], 1.0)

    # --- PSUM pools (8 banks: 4 sc + 2 ctx + 2 misc) ---
    pssc = ctx.enter_context(tc.tile_pool(name="pssc", bufs=2, space="PSUM"))
    psctx = ctx.enter_context(tc.tile_pool(name="psctx", bufs=1, space="PSUM"))
    psmisc = ctx.enter_context(tc.tile_pool(name="psmisc", bufs=2, space="PSUM"))

    # SBUF working pools
    expp = ctx.enter_context(tc.tile_pool(name="expp", bufs=5))
    cup = ctx.enter_context(tc.tile_pool(name="cup", bufs=6))
    bcp = ctx.enter_context(tc.tile_pool(name="bcp", bufs=8))
    outp = ctx.enter_context(tc.tile_pool(name="outp", bufs=2))

    # --- PE warmup during the DMA window: ramps the tensor engine to full
    # p-state before real work. Outputs are consumed to keep walrus honest.
    scratch = persist.tile([128, SBW], BF16, name="scratch")
    scon = persist.tile([128, SBW], F32, name="scon")
    nc.vector.memset(scratch[:], 0.0)
    for i in range(WARMUP):
        wp = psmisc.tile([128, SBW], F32, tag="misc")
        nc.tensor.matmul(wp[:], scratch[:, 0:128], scratch[:], start=True, stop=True)
        if i % 8 == 7:
            nc.vector.tensor_copy(scon[:], wp[:])

    # --- work units ---
    def q_unit(m, s):
        msl = slice(128 * m, 128 * (m + 1))
        ssl = slice(SBW * s, SBW * (s + 1))
        qp = psmisc.tile([128, SBW], F32, tag="misc")
        for e in range(NE):
            nc.tensor.matmul(
                qp[:], wq_sb[:, e, msl], hs_sb[:, e, ssl],
                start=(e == 0), stop=(e == NE - 1),
            )
        nc.vector.tensor_scalar_add(
            out=qT_sb[m][:, ssl], in0=qp[:], scalar1=bq_sb[:, m : m + 1]
        )

    def k_unit(m, s):
        msl = slice(128 * m, 128 * (m + 1))
        ssl = slice(SBW * s, SBW * (s + 1))
        kp = psmisc.tile([128, SBW], F32, tag="misc")
        for e in range(NE):
            nc.tensor.matmul(
                kp[:], wk_sb[:, e, msl], hs_sb[:, e, ssl],
                start=(e == 0), stop=(e == NE - 1),
            )
        nc.vector.tensor_scalar_add(
            out=kT_sb[m][:, ssl], in0=kp[:], scalar1=bk_sb[:, m : m + 1]
        )

    def v_unit(t):
        tsl = slice(128 * t, 128 * (t + 1))
        vp = psmisc.tile([128, SBW], F32, tag="misc")
        for e in range(NE):
            nc.tensor.matmul(
                vp[:, 0:HD], hs_sb[:, e, tsl], wv_sb[:, e, :],
                start=(e == 0), stop=(e == NE - 1),
            )
        nc.vector.tensor_add(
            out=v_aug[t][:, :, 32 : 32 + D],
            in0=vp[:, 0:HD].rearrange("p (h d) -> p h d", h=NH),
            in1=bv_bc[:].rearrange("p (h d) -> p h d", h=NH),
        )

    def out_unit(s, et):
        esl = slice(128 * et, 128 * (et + 1))
        ssl = slice(SBW * s, SBW * (s + 1))
        op = psmisc.tile([128, SBW], F32, tag="misc")
        for f in range(NM):
            nc.tensor.matmul(
                op[:], woT_sb[:, f, esl], ctxT_sb[f][:, ssl],
                start=(f == 0), stop=(f == NM - 1),
            )
        ob = outp.tile([128, SBW], F32, tag="ob")
        nc.vector.tensor_scalar_add(
            out=ob[:], in0=op[:], scalar1=bo_sb[:, et : et + 1]
        )
        nc.sync.dma_start(outT[esl, ssl], ob[:])

    # --- lead-in: just the two chains the first score tiles need; scores
    # for t-tiles 4k.. wait on k_unit(0, k) which streams in as background
    k_unit(0, 0)
    q_unit(0, 0)

    bg = deque()          # v tiles just-in-time: v[t] issued at step t
    for t in range(NT):
        bg.append(lambda t=t: v_unit(t))
    # bg2 in deadline order: chains needed during s-block 0 first (popped
    # every step of the first pair), then the slack q chains
    bg2 = deque()
    for s in range(1, NS):
        bg2.append(lambda s=s: k_unit(0, s))
    for m in (1, 2):
        bg2.append(lambda m=m: q_unit(m, 0))
        for s in range(NS):
            bg2.append(lambda m=m, s=s: k_unit(m, s))
    for s in range(1, NS):
        for m in range(NM):
            bg2.append(lambda m=m, s=s: q_unit(m, s))
    bgout = deque()       # fused out-proj units

    # --- phase 2 ---
    pending_norm = [None]  # deferred normalize-scale of the previous pair

    def norm_scale(p, ssl, cu2):
        """Z -> partition-broadcast (from cu row 0) -> fast reciprocal on the
        broadcast (partition-parallel, same cost) -> scale ctx into ctxT.
        No DMA hops; deferred into the NEXT pair so it never gates PSUM."""
        c = RECIP_APPROX_FAST_CONSTS
        for a in range(2):
            bcz = bcp.tile([VW, SBW], F32, tag="bcs")
            nc.gpsimd.partition_broadcast(bcz[:], cu2[a][0:1, :])
            bci = bcp.tile([VW, SBW], F32, tag="bcs")
            nc.vector._custom_dve(
                RECIPROCAL_APPROX_FAST, out=bci[:], in0=bcz[:],
                s0=c["s0"], s1=c["s1"], imm2=c["imm2"],
            )
            for q in range(2):  # 32-partition chunks (alignment rules)
                nc.vector.tensor_mul(
                    out=ctxT_sb[p][D * a + 32 * q : D * a + 32 * (q + 1), ssl],
                    in0=cu2[a][32 + 32 * q : 64 + 32 * q, :],
                    in1=bci[32 + 32 * q : 64 + 32 * q, :],
                )

    carry = [None]  # finishes the previous pair: flush last 2 ctx groups
    # (their exp deps are already satisfied), evacuate PSUM, queue norm

    for pi, (s, p) in enumerate([(s, p) for s in range(NS) for p in range(NM)]):
        ssl = slice(SBW * s, SBW * (s + 1))
        kTh = kT_sb[p]
        qTh = qT_sb[p]
        cps = []  # psctx tiles, allocated lazily at first ctx use (after
        # the previous pair's flush+evac have been issued)
        pend = deque()  # depth-2: ctx consumes exp from 2 steps ago, so
        # its semaphore wait is pre-satisfied (no PE stall)

        def ctx_mms(ex, t, stop, p=p, cps=cps):
            if not cps:
                cpA = psctx.tile([128, SBW], F32, tag="ctxA", name="cpA")
                cpB = psctx.tile([128, SBW], F32, tag="ctxB", name="cpB")
                cps.extend([cpA, cpB])
            st = t == 0
            nc.tensor.matmul(
                cps[0][0:VW, :], v_aug[t][:, 2 * p, :], ex[:, 0:SBW],
                start=st, stop=stop,
            )
            nc.tensor.matmul(
                cps[1][0:VW, :], v_aug[t][:, 2 * p + 1, :], ex[:, SBW : 2 * SBW],
                start=st, stop=stop,
            )

        for t in range(NT):
            tsl = slice(128 * t, 128 * (t + 1))
            if len(pend) == 2:
                ex2, t2 = pend.popleft()
                ctx_mms(ex2, t2, stop=False)
            sc = pssc.tile([128, 2 * SBW], F32, tag="sc")
            nc.tensor.matmul(
                sc[:, 0:SBW], kTh[0:D, tsl], qTh[0:D, ssl],
                start=True, stop=True,
            )
            nc.tensor.matmul(
                sc[:, SBW : 2 * SBW], kTh[D:128, tsl], qTh[D:128, ssl],
                start=True, stop=True,
            )
            ex = expp.tile([128, 2 * SBW], BF16, tag="exp")
            nc.scalar.activation(ex[:], sc[:], AF.Exp)
            pend.append((ex, t))
            if t == 1 and carry[0] is not None:
                carry[0]()
                carry[0] = None
            if t == 3 and pending_norm[0] is not None:
                pending_norm[0]()
                pending_norm[0] = None
            # background fill: v just-in-time; bg2 chains every step during
            # the first (heavy) pair, every other step after; out-proj in
            # the remaining slots
            if bg:
                bg.popleft()()
            if bg2 and (pi == 0 or t % 2 == 0) and t < 14:
                bg2.popleft()()
            elif bgout and (p > 0 or t >= 6) and t < 14:
                bgout.popleft()()

        def finish_pair(pend=pend, ctx_mms=ctx_mms, cps=cps, p=p, ssl=ssl):
            while pend:
                ex2, t2 = pend.popleft()
                ctx_mms(ex2, t2, stop=(t2 == NT - 1))
            cu2 = []
            for cp in cps:
                cu = cup.tile([VW, SBW], F32, tag="cu")
                nc.vector.tensor_copy(cu[:], cp[0:VW, :])
                cu2.append(cu)
            pending_norm[0] = lambda: norm_scale(p, ssl, cu2)

        carry[0] = finish_pair
        if p == NM - 1:
            for et in range(NE):
                bgout.append(lambda s=s, et=et: out_unit(s, et))

    # --- drain: final pair's flush + normalize, then remaining out-proj ---
    carry[0]()
    pending_norm[0]()
    while bg2:
        bg2.popleft()()
    while bgout:
        bgout.popleft()()


def build_nc():
    nc = bacc.Bacc("TRN2", target_bir_lowering=False, debug=False)
    hs = nc.dram_tensor("hs", [128, NE, S], BF16, kind="ExternalInput")
    wq = nc.dram_tensor("wq", [128, NE, HD], BF16, kind="ExternalInput")
    wk = nc.dram_tensor("wk", [128, NE, HD], BF16, kind="ExternalInput")
    wv = nc.dram_tensor("wv", [128, NE, HD], BF16, kind="ExternalInput")
    bq = nc.dram_tensor("bq", [HD], F32, kind="ExternalInput")
    bk = nc.dram_tensor("bk", [HD], F32, kind="ExternalInput")
    bv = nc.dram_tensor("bv", [HD], F32, kind="ExternalInput")
    woT = nc.dram_tensor("woT", [128, NM, E], BF16, kind="ExternalInput")
    bo2 = nc.dram_tensor("bo2", [E], F32, kind="ExternalInput")
    outT = nc.dram_tensor("outT", [E, S], F32, kind="ExternalOutput")

    with tile.TileContext(nc) as tc:
        mha_tile(
            tc,
            hs[:, :, :], wq[:, :, :], wk[:, :, :], wv[:, :, :],
            bq[:], bk[:], bv[:],
            woT[:, :, :], bo2[:], outT[:, :],
        )
    nc.compile()
    return nc


def _pack(x: np.ndarray, nchunk: int) -> np.ndarray:
    """[nchunk*128, cols] row-major -> [128, nchunk, cols] (partition-major)."""
    bf16 = ml_dtypes.bfloat16
    cols = x.shape[1]
    return np.ascontiguousarray(
        x.reshape(nchunk, 128, cols).transpose(1, 0, 2).astype(bf16)
    )


def make_core_inputs(inputs: dict) -> list[dict]:
    """Full inputs -> per-core input maps (core c: batch c//2, head-group c%2)."""
    hsf = np.ascontiguousarray(np.asarray(inputs["hidden_state"], dtype=np.float32))
    Wq = np.asarray(inputs["Wq"], dtype=np.float32)
    Wk = np.asarray(inputs["Wk"], dtype=np.float32)
    Wv = np.asarray(inputs["Wv"], dtype=np.float32)
    Wo = np.asarray(inputs["Wo"], dtype=np.float32)
    bq = np.asarray(inputs["bq"], dtype=np.float32)
    bk = np.asarray(inputs["bk"], dtype=np.float32)
    bv = np.asarray(inputs["bv"], dtype=np.float32)
    bo = np.asarray(inputs["bo"], dtype=np.float32)

    maps = []
    for c in range(8):
        b, g = c // 2, c % 2
        hsl = slice(NH * g, NH * (g + 1))
        fsl = slice(HD * g, HD * (g + 1))
        maps.append(
            {
                "hs": _pack(hsf[b], NE),
                "wq": _pack(Wq[hsl].transpose(1, 0, 2).reshape(E, HD), NE),
                "wk": _pack(Wk[hsl].transpose(1, 0, 2).reshape(E, HD), NE),
                "wv": _pack(Wv[hsl].transpose(1, 0, 2).reshape(E, HD), NE),
                "bq": np.ascontiguousarray(bq[hsl].reshape(HD)),
                "bk": np.ascontiguousarray(bk[hsl].reshape(HD)),
                "bv": np.ascontiguousarray(bv[hsl].reshape(HD)),
                "woT": _pack(np.ascontiguousarray(Wo[:, fsl].T), NM),
                "bo2": np.ascontiguousarray(bo / 2.0),
            }
        )
    return maps


def combine_outputs(core_outs: list) -> np.ndarray:
    """Per-core outT partials -> full [B, E, S] output."""
    return np.stack(
        [core_outs[2 * b]["outT"] + core_outs[2 * b + 1]["outT"] for b in range(B)]
    ).astype(np.float32)


from concourse.bass_utils import run_bass_kernel_spmd

N_CORES = 8
_NC_CACHE = None


def _get_nc():
    global _NC_CACHE
    if _NC_CACHE is None:
        _NC_CACHE = build_nc()
    return _NC_CACHE


def kernel(**inputs) -> np.ndarray:
    """Full-input entry point: shard across 8 cores, run, unshard."""
    maps = make_core_inputs(inputs)
    nc = _get_nc()
    res = run_bass_kernel_spmd(nc, maps, core_ids=list(range(N_CORES)))
    outs = res.results
    return np.stack(
        [outs[2 * b]["outT"] + outs[2 * b + 1]["outT"] for b in range(B)]
    ).astype(np.float32)


# revision 55
# speedup vs baseline: 1.0220x; 1.0086x over previous
"""Multi-head attention Bass/Tile kernel for TRN2, 8-core SPMD.

Sharding: core c handles batch b = c//2 and head-group g = c%2 (6 of 12 heads).
Each core computes its 6 heads end-to-end plus a partial output projection
(over its 384 of 768 ctx dims); the host sums the two partials per batch.

Design: all matmuls bf16 (PSUM accumulation stays f32). The ScalarE exp
stream (192 x [128,1024] ~ 208us) is the hard floor; everything else hides
under it on a dense PE queue:
  - inputs packed host-side as [128, chunk, cols] so each partition's data
    is one contiguous DRAM range (few, large DMA descriptors)
  - PE warmup matmuls during the DMA window (p-state ramps 0.65->2.4 GHz)
  - lead-in computes only m0 qT/kT; v tiles are produced just-in-time and
    m1/m2 q/k chains run as background units inside the attention loop
  - score pairs K=64 run row-tiled (concurrent halves), one exp instruction
    covers both heads of a pair, ctx accumulates pend-by-1
  - per-pair softmax normalization: one [2,512] reciprocal, gpsimd
    partition-broadcast, DVE multiplies
  - out-proj fused per s-block into the next block's background slots
"""

from collections import deque
from contextlib import ExitStack

import numpy as np
import ml_dtypes

import concourse.bass as bass
import concourse.tile as tile
from concourse import bacc, mybir
from concourse._compat import with_exitstack
from concourse.dve_ops import RECIPROCAL_APPROX_FAST, RECIP_APPROX_FAST_CONSTS

F32 = mybir.dt.float32
BF16 = mybir.dt.bfloat16
AF = mybir.ActivationFunctionType

B, E, S, H, D = 4, 768, 2048, 12, 64
NH = 6          # heads per core
HD = NH * D     # 384 head-dims per core
NE = E // 128   # 6 e-chunks
NM = HD // 128  # 3 m-chunks (2 heads each)
NT = S // 128   # 16 t-tiles
SBW = 512       # s-block width
NS = S // SBW   # 4 s-blocks
VW = 96         # ctx stationary width: col 0 = ones (denominator), 32:96 = v
WARMUP = 24


@with_exitstack
def mha_tile(ctx: ExitStack, tc, hs, wq, wk, wv, bq, bk, bv, woT, bo2, outT):
    nc = tc.nc

    persist = ctx.enter_context(tc.tile_pool(name="persist", bufs=1))

    # --- persistent SBUF tiles (packed layouts match the DRAM packing) ---
    hs_sb = persist.tile([128, NE, S], BF16, name="hs")
    wq_sb = persist.tile([128, NE, HD], BF16, name="wq")
    wk_sb = persist.tile([128, NE, HD], BF16, name="wk")
    wv_sb = persist.tile([128, NE, HD], BF16, name="wv")
    woT_sb = persist.tile([128, NM, E], BF16, name="wo")
    qT_sb = [persist.tile([128, S], BF16, name=f"qT{m}") for m in range(NM)]
    kT_sb = [persist.tile([128, S], BF16, name=f"kT{m}") for m in range(NM)]
    ctxT_sb = [persist.tile([128, S], BF16, name=f"ctxT{m}") for m in range(NM)]
    v_aug = [persist.tile([128, NH, VW], BF16, name=f"vaug{t}") for t in range(NT)]


    bq_sb = persist.tile([128, NM], F32, name="bq")
    bk_sb = persist.tile([128, NM], F32, name="bk")
    bv_bc = persist.tile([128, HD], F32, name="bv")
    bo_sb = persist.tile([128, NE], F32, name="bo")

    # --- DMA: large contiguous-per-partition transfers, split in halves so
    # two queues run in parallel per tensor ---
    nc.sync.dma_start(hs_sb[:], hs)
    nc.sync.dma_start(wq_sb[:], wq)
    nc.sync.dma_start(wk_sb[:], wk)
    nc.sync.dma_start(wv_sb[:], wv)
    nc.sync.dma_start(woT_sb[:], woT)
    nc.sync.dma_start(bq_sb[:], bq.rearrange("(m p) -> p m", p=128))
    nc.sync.dma_start(bk_sb[:], bk.rearrange("(m p) -> p m", p=128))
    nc.sync.dma_start(
        bv_bc[:], bass.AP(tensor=bv.tensor, offset=bv.offset, ap=[[0, 128], [1, HD]])
    )
    nc.sync.dma_start(bo_sb[:], bo2.rearrange("(m p) -> p m", p=128))

    # v_aug init on gpsimd (vector stays free): zeros, then ones in col 0
    for t in range(NT):
        nc.gpsimd.memset(v_aug[t][:].rearrange("p h d -> p (h d)"), 0.0)
        nc.gpsimd.memset(v_aug[t][:, :, 0:1], 1.0)

    # --- PSUM pools (8 banks: 4 sc + 2 ctx + 2 misc) ---
    pssc = ctx.enter_context(tc.tile_pool(name="pssc", bufs=2, space="PSUM"))
    psctx = ctx.enter_context(tc.tile_pool(name="psctx", bufs=1, space="PSUM"))
    psmisc = ctx.enter_context(tc.tile_pool(name="psmisc", bufs=2, space="PSUM"))

    # SBUF working pools
    expp = ctx.enter_context(tc.tile_pool(name="expp", bufs=5))
    cup = ctx.enter_context(tc.tile_pool(name="cup", bufs=6))
    bcp = ctx.enter_context(tc.tile_pool(name="bcp", bufs=8))
    outp = ctx.enter_context(tc.tile_pool(name="outp", bufs=2))

    # --- PE warmup during the DMA window: ramps the tensor engine to full
    # p-state before real work. Outputs are consumed to keep walrus honest.
    scratch = persist.tile([128, SBW], BF16, name="scratch")
    scon = persist.tile([128, SBW], F32, name="scon")
    nc.vector.memset(scratch[:], 0.0)
    for i in range(WARMUP):
        wp = psmisc.tile([128, SBW], F32, tag="misc")
        nc.tensor.matmul(wp[:], scratch[:, 0:128], scratch[:], start=True, stop=True)
        if i % 8 == 7:
            nc.vector.tensor_copy(scon[:], wp[:])

    # --- work units ---
    def q_unit(m, s):
        msl = slice(128 * m, 128 * (m + 1))
        ssl = slice(SBW * s, SBW * (s + 1))
        qp = psmisc.tile([128, SBW], F32, tag="misc")
        for e in range(NE):
            nc.tensor.matmul(
                qp[:], wq_sb[:, e, msl], hs_sb[:, e, ssl],
                start=(e == 0), stop=(e == NE - 1),
            )
        nc.vector.tensor_scalar_add(
            out=qT_sb[m][:, ssl], in0=qp[:], scalar1=bq_sb[:, m : m + 1]
        )

    def k_unit(m, s):
        msl = slice(128 * m, 128 * (m + 1))
        ssl = slice(SBW * s, SBW * (s + 1))
        kp = psmisc.tile([128, SBW], F32, tag="misc")
        for e in range(NE):
            nc.tensor.matmul(
                kp[:], wk_sb[:, e, msl], hs_sb[:, e, ssl],
                start=(e == 0), stop=(e == NE - 1),
            )
        nc.vector.tensor_scalar_add(
            out=kT_sb[m][:, ssl], in0=kp[:], scalar1=bk_sb[:, m : m + 1]
        )

    def v_unit(t):
        tsl = slice(128 * t, 128 * (t + 1))
        vp = psmisc.tile([128, SBW], F32, tag="misc")
        for e in range(NE):
            nc.tensor.matmul(
                vp[:, 0:HD], hs_sb[:, e, tsl], wv_sb[:, e, :],
                start=(e == 0), stop=(e == NE - 1),
            )
        nc.vector.tensor_add(
            out=v_aug[t][:, :, 32 : 32 + D],
            in0=vp[:, 0:HD].rearrange("p (h d) -> p h d", h=NH),
            in1=bv_bc[:].rearrange("p (h d) -> p h d", h=NH),
        )

    def out_unit(s, et):
        esl = slice(128 * et, 128 * (et + 1))
        ssl = slice(SBW * s, SBW * (s + 1))
        op = psmisc.tile([128, SBW], F32, tag="misc")
        for f in range(NM):
            nc.tensor.matmul(
                op[:], woT_sb[:, f, esl], ctxT_sb[f][:, ssl],
                start=(f == 0), stop=(f == NM - 1),
            )
        ob = outp.tile([128, SBW], F32, tag="ob")
        nc.vector.tensor_scalar_add(
            out=ob[:], in0=op[:], scalar1=bo_sb[:, et : et + 1]
        )
        nc.sync.dma_start(outT[esl, ssl], ob[:])

    # --- lead-in: just the two chains the first score tiles need; scores
    # for t-tiles 4k.. wait on k_unit(0, k) which streams in as background
    k_unit(0, 0)
    q_unit(0, 0)

    bg = deque()          # v tiles just-in-time: v[t] issued at step t
    for t in range(NT):
        bg.append(lambda t=t: v_unit(t))
    # bg2 in deadline order for the p-outer sweep: m0's remaining chains
    # (pairs 1-3), then kT[1]+qT[1] (pair 4), then the rest
    bg2 = deque()
    for s in range(1, NS):
        bg2.append(lambda s=s: k_unit(0, s))
        bg2.append(lambda s=s: q_unit(0, s))
    for m in (1, 2):
        for s in range(NS):
            bg2.append(lambda m=m, s=s: k_unit(m, s))
        for s in range(NS):
            bg2.append(lambda m=m, s=s: q_unit(m, s))
    bgout = deque()       # fused out-proj units

    # --- phase 2 ---
    pending_norm = [None]  # deferred normalize-scale of the previous pair

    def norm_scale(p, ssl, cu2):
        """Z -> partition-broadcast (from cu row 0) -> fast reciprocal on the
        broadcast (partition-parallel, same cost) -> scale ctx into ctxT.
        No DMA hops; deferred into the NEXT pair so it never gates PSUM."""
        c = RECIP_APPROX_FAST_CONSTS
        for a in range(2):
            bcz = bcp.tile([VW, SBW], F32, tag="bcs")
            nc.gpsimd.partition_broadcast(bcz[:], cu2[a][0:1, :])
            bci = bcp.tile([VW, SBW], F32, tag="bcs")
            nc.vector._custom_dve(
                RECIPROCAL_APPROX_FAST, out=bci[:], in0=bcz[:],
                s0=c["s0"], s1=c["s1"], imm2=c["imm2"],
            )
            for q in range(2):  # 32-partition chunks (alignment rules)
                nc.vector.tensor_mul(
                    out=ctxT_sb[p][D * a + 32 * q : D * a + 32 * (q + 1), ssl],
                    in0=cu2[a][32 + 32 * q : 64 + 32 * q, :],
                    in1=bci[32 + 32 * q : 64 + 32 * q, :],
                )

    carry = [None]  # finishes the previous pair: flush last 2 ctx groups
    # (their exp deps are already satisfied), evacuate PSUM, queue norm

    # p-outer sweep: head-pair deadlines for background qk chains land 4x
    # later than s-outer, so the chain work spreads across the whole span
    for pi, (s, p) in enumerate([(s, p) for p in range(NM) for s in range(NS)]):
        ssl = slice(SBW * s, SBW * (s + 1))
        kTh = kT_sb[p]
        qTh = qT_sb[p]
        cps = []  # psctx tiles, allocated lazily at first ctx use (after
        # the previous pair's flush+evac have been issued)
        pend = deque()  # depth-2: ctx consumes exp from 2 steps ago, so
        # its semaphore wait is pre-satisfied (no PE stall)

        def ctx_mms(ex, t, stop, p=p, cps=cps):
            if not cps:
                cpA = psctx.tile([128, SBW], F32, tag="ctxA", name="cpA")
                cpB = psctx.tile([128, SBW], F32, tag="ctxB", name="cpB")
                cps.extend([cpA, cpB])
            st = t == 0
            nc.tensor.matmul(
                cps[0][0:VW, :], v_aug[t][:, 2 * p, :], ex[:, 0:SBW],
                start=st, stop=stop,
            )
            nc.tensor.matmul(
                cps[1][0:VW, :], v_aug[t][:, 2 * p + 1, :], ex[:, SBW : 2 * SBW],
                start=st, stop=stop,
            )

        for t in range(NT):
            tsl = slice(128 * t, 128 * (t + 1))
            if len(pend) == 2:
                ex2, t2 = pend.popleft()
                ctx_mms(ex2, t2, stop=False)
            sc = pssc.tile([128, 2 * SBW], F32, tag="sc")
            nc.tensor.matmul(
                sc[:, 0:SBW], kTh[0:D, tsl], qTh[0:D, ssl],
                start=True, stop=True,
            )
            nc.tensor.matmul(
                sc[:, SBW : 2 * SBW], kTh[D:128, tsl], qTh[D:128, ssl],
                start=True, stop=True,
            )
            ex = expp.tile([128, 2 * SBW], BF16, tag="exp")
            nc.scalar.activation(ex[:], sc[:], AF.Exp)
            pend.append((ex, t))
            if t == 1 and carry[0] is not None:
                carry[0]()
                carry[0] = None
            if t == 3 and pending_norm[0] is not None:
                pending_norm[0]()
                pending_norm[0] = None
            # background fill: v just-in-time; bg2 chains every step during
            # the first (heavy) pair, every other step after; out-proj in
            # the remaining slots
            if bg:
                bg.popleft()()
            if bg2 and t % 2 == 0 and t < 14:
                bg2.popleft()()
            elif bgout and 6 <= t < 14:  # after this pair's t==3 norm_scale
                bgout.popleft()()

        def finish_pair(pend=pend, ctx_mms=ctx_mms, cps=cps, p=p, ssl=ssl):
            while pend:
                ex2, t2 = pend.popleft()
                ctx_mms(ex2, t2, stop=(t2 == NT - 1))
            cu2 = []
            for cp in cps:
                cu = cup.tile([VW, SBW], F32, tag="cu")
                nc.vector.tensor_copy(cu[:], cp[0:VW, :])
                cu2.append(cu)
            pending_norm[0] = lambda: norm_scale(p, ssl, cu2)

        carry[0] = finish_pair
        if p == NM - 1:
            for et in range(NE):
                bgout.append(lambda s=s, et=et: out_unit(s, et))

    # --- drain: final pair's flush + normalize, then remaining out-proj ---
    carry[0]()
    pending_norm[0]()
    while bg2:
        bg2.popleft()()
    while bgout:
        bgout.popleft()()


def build_nc():
    nc = bacc.Bacc("TRN2", target_bir_lowering=False, debug=False)
    hs = nc.dram_tensor("hs", [128, NE, S], BF16, kind="ExternalInput")
    wq = nc.dram_tensor("wq", [128, NE, HD], BF16, kind="ExternalInput")
    wk = nc.dram_tensor("wk", [128, NE, HD], BF16, kind="ExternalInput")
    wv = nc.dram_tensor("wv", [128, NE, HD], BF16, kind="ExternalInput")
    bq = nc.dram_tensor("bq", [HD], F32, kind="ExternalInput")
    bk = nc.dram_tensor("bk", [HD], F32, kind="ExternalInput")
    bv = nc.dram_tensor("bv", [HD], F32, kind="ExternalInput")
    woT = nc.dram_tensor("woT", [128, NM, E], BF16, kind="ExternalInput")
    bo2 = nc.dram_tensor("bo2", [E], F32, kind="ExternalInput")
    outT = nc.dram_tensor("outT", [E, S], F32, kind="ExternalOutput")

    with tile.TileContext(nc) as tc:
        mha_tile(
            tc,
            hs[:, :, :], wq[:, :, :], wk[:, :, :], wv[:, :, :],
            bq[:], bk[:], bv[:],
            woT[:, :, :], bo2[:], outT[:, :],
        )
    nc.compile()
    return nc


def _pack(x: np.ndarray, nchunk: int) -> np.ndarray:
    """[nchunk*128, cols] row-major -> [128, nchunk, cols] (partition-major)."""
    bf16 = ml_dtypes.bfloat16
    cols = x.shape[1]
    return np.ascontiguousarray(
        x.reshape(nchunk, 128, cols).transpose(1, 0, 2).astype(bf16)
    )


def make_core_inputs(inputs: dict) -> list[dict]:
    """Full inputs -> per-core input maps (core c: batch c//2, head-group c%2)."""
    hsf = np.ascontiguousarray(np.asarray(inputs["hidden_state"], dtype=np.float32))
    Wq = np.asarray(inputs["Wq"], dtype=np.float32)
    Wk = np.asarray(inputs["Wk"], dtype=np.float32)
    Wv = np.asarray(inputs["Wv"], dtype=np.float32)
    Wo = np.asarray(inputs["Wo"], dtype=np.float32)
    bq = np.asarray(inputs["bq"], dtype=np.float32)
    bk = np.asarray(inputs["bk"], dtype=np.float32)
    bv = np.asarray(inputs["bv"], dtype=np.float32)
    bo = np.asarray(inputs["bo"], dtype=np.float32)

    maps = []
    for c in range(8):
        b, g = c // 2, c % 2
        hsl = slice(NH * g, NH * (g + 1))
        fsl = slice(HD * g, HD * (g + 1))
        maps.append(
            {
                "hs": _pack(hsf[b], NE),
                "wq": _pack(Wq[hsl].transpose(1, 0, 2).reshape(E, HD), NE),
                "wk": _pack(Wk[hsl].transpose(1, 0, 2).reshape(E, HD), NE),
                "wv": _pack(Wv[hsl].transpose(1, 0, 2).reshape(E, HD), NE),
                "bq": np.ascontiguousarray(bq[hsl].reshape(HD)),
                "bk": np.ascontiguousarray(bk[hsl].reshape(HD)),
                "bv": np.ascontiguousarray(bv[hsl].reshape(HD)),
                "woT": _pack(np.ascontiguousarray(Wo[:, fsl].T), NM),
                "bo2": np.ascontiguousarray(bo / 2.0),
            }
        )
    return maps


def combine_outputs(core_outs: list) -> np.ndarray:
    """Per-core outT partials -> full [B, E, S] output."""
    return np.stack(
        [core_outs[2 * b]["outT"] + core_outs[2 * b + 1]["outT"] for b in range(B)]
    ).astype(np.float32)


from concourse.bass_utils import run_bass_kernel_spmd

N_CORES = 8
_NC_CACHE = None


def _get_nc():
    global _NC_CACHE
    if _NC_CACHE is None:
        _NC_CACHE = build_nc()
    return _NC_CACHE


def kernel(**inputs) -> np.ndarray:
    """Full-input entry point: shard across 8 cores, run, unshard."""
    maps = make_core_inputs(inputs)
    nc = _get_nc()
    res = run_bass_kernel_spmd(nc, maps, core_ids=list(range(N_CORES)))
    outs = res.results
    return np.stack(
        [outs[2 * b]["outT"] + outs[2 * b + 1]["outT"] for b in range(B)]
    ).astype(np.float32)


# revision 63
# speedup vs baseline: 1.0264x; 1.0043x over previous
"""Multi-head attention Bass/Tile kernel for TRN2, 8-core SPMD.

Sharding: core c handles batch b = c//2 and head-group g = c%2 (6 of 12 heads).
Each core computes its 6 heads end-to-end plus a partial output projection
(over its 384 of 768 ctx dims); the host sums the two partials per batch.

Design: all matmuls bf16 (PSUM accumulation stays f32). The ScalarE exp
stream (192 x [128,1024] ~ 208us) is the hard floor; everything else hides
under it on a dense PE queue:
  - inputs packed host-side as [128, chunk, cols] so each partition's data
    is one contiguous DRAM range (few, large DMA descriptors)
  - PE warmup matmuls during the DMA window (p-state ramps 0.65->2.4 GHz)
  - lead-in computes only m0 qT/kT; v tiles are produced just-in-time and
    m1/m2 q/k chains run as background units inside the attention loop
  - score pairs K=64 run row-tiled (concurrent halves), one exp instruction
    covers both heads of a pair, ctx accumulates pend-by-1
  - per-pair softmax normalization: one [2,512] reciprocal, gpsimd
    partition-broadcast, DVE multiplies
  - out-proj fused per s-block into the next block's background slots
"""

from collections import deque
from contextlib import ExitStack

import numpy as np
import ml_dtypes

import concourse.bass as bass
import concourse.tile as tile
from concourse import bacc, mybir
from concourse._compat import with_exitstack
from concourse.dve_ops import RECIPROCAL_APPROX_FAST, RECIP_APPROX_FAST_CONSTS

F32 = mybir.dt.float32
BF16 = mybir.dt.bfloat16
AF = mybir.ActivationFunctionType

B, E, S, H, D = 4, 768, 2048, 12, 64
NH = 6          # heads per core
HD = NH * D     # 384 head-dims per core
NE = E // 128   # 6 e-chunks
NM = HD // 128  # 3 m-chunks (2 heads each)
NT = S // 128   # 16 t-tiles
SBW = 512       # s-block width
NS = S // SBW   # 4 s-blocks
VW = 96         # ctx stationary width: col 0 = ones (denominator), 32:96 = v
WARMUP = 24


@with_exitstack
def mha_tile(ctx: ExitStack, tc, hs, wq, wk, wv, bq, bk, bv, woT, bo2, outT):
    nc = tc.nc

    persist = ctx.enter_context(tc.tile_pool(name="persist", bufs=1))

    # --- persistent SBUF tiles (packed layouts match the DRAM packing);
    # hs is s-chunked: the first score/exp work only needs chunk 0, so the
    # lead-in waits on ~2MB of DMA instead of the full 5.5MB ---
    hs_sb = [persist.tile([128, NE, SBW], BF16, name=f"hs{c}") for c in range(NS)]
    wq_sb = persist.tile([128, NE, HD], BF16, name="wq")
    wk_sb = persist.tile([128, NE, HD], BF16, name="wk")
    wv_sb = persist.tile([128, NE, HD], BF16, name="wv")
    woT_sb = persist.tile([128, NM, E], BF16, name="wo")
    qT_sb = [persist.tile([128, S], BF16, name=f"qT{m}") for m in range(NM)]
    kT_sb = [persist.tile([128, S], BF16, name=f"kT{m}") for m in range(NM)]
    ctxT_sb = [persist.tile([128, S], BF16, name=f"ctxT{m}") for m in range(NM)]
    v_aug = [persist.tile([128, NH, VW], BF16, name=f"vaug{t}") for t in range(NT)]


    bq_sb = persist.tile([128, NM], F32, name="bq")
    bk_sb = persist.tile([128, NM], F32, name="bk")
    bv_bc = persist.tile([128, HD], F32, name="bv")
    bo_sb = persist.tile([128, NE], F32, name="bo")

    # --- DMA: large contiguous-per-partition transfers, split in halves so
    # two queues run in parallel per tensor ---
    nc.sync.dma_start(hs_sb[0][:], hs[:, 0, :, :])
    nc.sync.dma_start(wq_sb[:], wq)
    nc.sync.dma_start(wk_sb[:], wk)
    for c in range(1, NS):
        nc.sync.dma_start(hs_sb[c][:], hs[:, c, :, :])
    nc.sync.dma_start(wv_sb[:], wv)
    nc.sync.dma_start(woT_sb[:], woT)
    nc.sync.dma_start(bq_sb[:], bq.rearrange("(m p) -> p m", p=128))
    nc.sync.dma_start(bk_sb[:], bk.rearrange("(m p) -> p m", p=128))
    nc.sync.dma_start(
        bv_bc[:], bass.AP(tensor=bv.tensor, offset=bv.offset, ap=[[0, 128], [1, HD]])
    )
    nc.sync.dma_start(bo_sb[:], bo2.rearrange("(m p) -> p m", p=128))

    # v_aug init on gpsimd (vector stays free): zeros, then ones in col 0
    for t in range(NT):
        nc.gpsimd.memset(v_aug[t][:].rearrange("p h d -> p (h d)"), 0.0)
        nc.gpsimd.memset(v_aug[t][:, :, 0:1], 1.0)

    # --- PSUM pools (8 banks: 4 sc + 2 ctx + 2 misc) ---
    pssc = ctx.enter_context(tc.tile_pool(name="pssc", bufs=2, space="PSUM"))
    psctx = ctx.enter_context(tc.tile_pool(name="psctx", bufs=1, space="PSUM"))
    psmisc = ctx.enter_context(tc.tile_pool(name="psmisc", bufs=2, space="PSUM"))

    # SBUF working pools
    expp = ctx.enter_context(tc.tile_pool(name="expp", bufs=5))
    cup = ctx.enter_context(tc.tile_pool(name="cup", bufs=6))
    bcp = ctx.enter_context(tc.tile_pool(name="bcp", bufs=8))
    outp = ctx.enter_context(tc.tile_pool(name="outp", bufs=2))

    # --- PE warmup during the DMA window: ramps the tensor engine to full
    # p-state before real work. Outputs are consumed to keep walrus honest.
    scratch = persist.tile([128, SBW], BF16, name="scratch")
    scon = persist.tile([128, SBW], F32, name="scon")
    nc.vector.memset(scratch[:], 0.0)
    for i in range(WARMUP):
        wp = psmisc.tile([128, SBW], F32, tag="misc")
        nc.tensor.matmul(wp[:], scratch[:, 0:128], scratch[:], start=True, stop=True)
        if i % 8 == 7:
            nc.vector.tensor_copy(scon[:], wp[:])

    # --- work units ---
    def q_unit(m, s):
        msl = slice(128 * m, 128 * (m + 1))
        ssl = slice(SBW * s, SBW * (s + 1))
        qp = psmisc.tile([128, SBW], F32, tag="misc")
        for e in range(NE):
            nc.tensor.matmul(
                qp[:], wq_sb[:, e, msl], hs_sb[s][:, e, :],
                start=(e == 0), stop=(e == NE - 1),
            )
        nc.vector.tensor_scalar_add(
            out=qT_sb[m][:, ssl], in0=qp[:], scalar1=bq_sb[:, m : m + 1]
        )

    def k_unit(m, s):
        msl = slice(128 * m, 128 * (m + 1))
        ssl = slice(SBW * s, SBW * (s + 1))
        kp = psmisc.tile([128, SBW], F32, tag="misc")
        for e in range(NE):
            nc.tensor.matmul(
                kp[:], wk_sb[:, e, msl], hs_sb[s][:, e, :],
                start=(e == 0), stop=(e == NE - 1),
            )
        nc.vector.tensor_scalar_add(
            out=kT_sb[m][:, ssl], in0=kp[:], scalar1=bk_sb[:, m : m + 1]
        )

    def v_unit(t):
        tsl = slice(128 * (t % 4), 128 * (t % 4 + 1))
        vp = psmisc.tile([128, SBW], F32, tag="misc")
        for e in range(NE):
            nc.tensor.matmul(
                vp[:, 0:HD], hs_sb[t // 4][:, e, tsl], wv_sb[:, e, :],
                start=(e == 0), stop=(e == NE - 1),
            )
        nc.vector.tensor_add(
            out=v_aug[t][:, :, 32 : 32 + D],
            in0=vp[:, 0:HD].rearrange("p (h d) -> p h d", h=NH),
            in1=bv_bc[:].rearrange("p (h d) -> p h d", h=NH),
        )

    def out_unit(s, et):
        esl = slice(128 * et, 128 * (et + 1))
        ssl = slice(SBW * s, SBW * (s + 1))
        op = psmisc.tile([128, SBW], F32, tag="misc")
        for f in range(NM):
            nc.tensor.matmul(
                op[:], woT_sb[:, f, esl], ctxT_sb[f][:, ssl],
                start=(f == 0), stop=(f == NM - 1),
            )
        ob = outp.tile([128, SBW], F32, tag="ob")
        nc.vector.tensor_scalar_add(
            out=ob[:], in0=op[:], scalar1=bo_sb[:, et : et + 1]
        )
        nc.sync.dma_start(outT[esl, ssl], ob[:])

    # --- lead-in: just the two chains the first score tiles need; scores
    # for t-tiles 4k.. wait on k_unit(0, k) which streams in as background
    k_unit(0, 0)
    q_unit(0, 0)

    bg = deque()          # v tiles just-in-time: v[t] issued at step t
    for t in range(NT):
        bg.append(lambda t=t: v_unit(t))
    # bg2 in deadline order for the p-outer sweep: m0's remaining chains
    # (pairs 1-3), then kT[1]+qT[1] (pair 4), then the rest
    bg2 = deque()
    for s in range(1, NS):
        bg2.append(lambda s=s: k_unit(0, s))
        bg2.append(lambda s=s: q_unit(0, s))
    for m in (1, 2):
        for s in range(NS):
            bg2.append(lambda m=m, s=s: k_unit(m, s))
        for s in range(NS):
            bg2.append(lambda m=m, s=s: q_unit(m, s))
    bgout = deque()       # fused out-proj units

    # --- phase 2 ---
    pending_norm = [None]  # deferred normalize-scale of the previous pair

    def norm_scale(p, ssl, cu2):
        """Z -> partition-broadcast (from cu row 0) -> fast reciprocal on the
        broadcast (partition-parallel, same cost) -> scale ctx into ctxT.
        No DMA hops; deferred into the NEXT pair so it never gates PSUM."""
        c = RECIP_APPROX_FAST_CONSTS
        for a in range(2):
            bcz = bcp.tile([VW, SBW], F32, tag="bcs")
            nc.gpsimd.partition_broadcast(bcz[:], cu2[a][0:1, :])
            bci = bcp.tile([VW, SBW], F32, tag="bcs")
            nc.vector._custom_dve(
                RECIPROCAL_APPROX_FAST, out=bci[:], in0=bcz[:],
                s0=c["s0"], s1=c["s1"], imm2=c["imm2"],
            )
            for q in range(2):  # 32-partition chunks (alignment rules)
                nc.vector.tensor_mul(
                    out=ctxT_sb[p][D * a + 32 * q : D * a + 32 * (q + 1), ssl],
                    in0=cu2[a][32 + 32 * q : 64 + 32 * q, :],
                    in1=bci[32 + 32 * q : 64 + 32 * q, :],
                )

    carry = [None]  # finishes the previous pair: flush last 2 ctx groups
    # (their exp deps are already satisfied), evacuate PSUM, queue norm

    # p-outer sweep: head-pair deadlines for background qk chains land 4x
    # later than s-outer, so the chain work spreads across the whole span
    for pi, (s, p) in enumerate([(s, p) for p in range(NM) for s in range(NS)]):
        ssl = slice(SBW * s, SBW * (s + 1))
        kTh = kT_sb[p]
        qTh = qT_sb[p]
        cps = []  # psctx tiles, allocated lazily at first ctx use (after
        # the previous pair's flush+evac have been issued)
        pend = deque()  # depth-2: ctx consumes exp from 2 steps ago, so
        # its semaphore wait is pre-satisfied (no PE stall)

        def ctx_mms(ex, t, stop, p=p, cps=cps):
            if not cps:
                cpA = psctx.tile([128, SBW], F32, tag="ctxA", name="cpA")
                cpB = psctx.tile([128, SBW], F32, tag="ctxB", name="cpB")
                cps.extend([cpA, cpB])
            st = t == 0
            nc.tensor.matmul(
                cps[0][0:VW, :], v_aug[t][:, 2 * p, :], ex[:, 0:SBW],
                start=st, stop=stop,
            )
            nc.tensor.matmul(
                cps[1][0:VW, :], v_aug[t][:, 2 * p + 1, :], ex[:, SBW : 2 * SBW],
                start=st, stop=stop,
            )

        for t in range(NT):
            tsl = slice(128 * t, 128 * (t + 1))
            if len(pend) == 2:
                ex2, t2 = pend.popleft()
                ctx_mms(ex2, t2, stop=False)
            sc = pssc.tile([128, 2 * SBW], F32, tag="sc")
            nc.tensor.matmul(
                sc[:, 0:SBW], kTh[0:D, tsl], qTh[0:D, ssl],
                start=True, stop=True,
            )
            nc.tensor.matmul(
                sc[:, SBW : 2 * SBW], kTh[D:128, tsl], qTh[D:128, ssl],
                start=True, stop=True,
            )
            ex = expp.tile([128, 2 * SBW], BF16, tag="exp")
            nc.scalar.activation(ex[:], sc[:], AF.Exp)
            pend.append((ex, t))
            if t == 1 and carry[0] is not None:
                carry[0]()
                carry[0] = None
            if t == 3 and pending_norm[0] is not None:
                pending_norm[0]()
                pending_norm[0] = None
            # background fill: v just-in-time; bg2 chains every step during
            # the first (heavy) pair, every other step after; out-proj in
            # the remaining slots
            if bg:
                bg.popleft()()
            if bg2 and t % 2 == 0 and t < 14:
                bg2.popleft()()
            elif bgout and 6 <= t < 14:  # after this pair's t==3 norm_scale
                bgout.popleft()()

        def finish_pair(pend=pend, ctx_mms=ctx_mms, cps=cps, p=p, ssl=ssl):
            while pend:
                ex2, t2 = pend.popleft()
                ctx_mms(ex2, t2, stop=(t2 == NT - 1))
            cu2 = []
            for cp in cps:
                cu = cup.tile([VW, SBW], F32, tag="cu")
                nc.vector.tensor_copy(cu[:], cp[0:VW, :])
                cu2.append(cu)
            pending_norm[0] = lambda: norm_scale(p, ssl, cu2)

        carry[0] = finish_pair
        if p == NM - 1:
            for et in range(NE):
                bgout.append(lambda s=s, et=et: out_unit(s, et))

    # --- drain: final pair's flush + normalize, then remaining out-proj ---
    carry[0]()
    pending_norm[0]()
    while bg2:
        bg2.popleft()()
    while bgout:
        bgout.popleft()()


def build_nc():
    nc = bacc.Bacc("TRN2", target_bir_lowering=False, debug=False)
    hs = nc.dram_tensor("hs", [128, NS, NE, SBW], BF16, kind="ExternalInput")
    wq = nc.dram_tensor("wq", [128, NE, HD], BF16, kind="ExternalInput")
    wk = nc.dram_tensor("wk", [128, NE, HD], BF16, kind="ExternalInput")
    wv = nc.dram_tensor("wv", [128, NE, HD], BF16, kind="ExternalInput")
    bq = nc.dram_tensor("bq", [HD], F32, kind="ExternalInput")
    bk = nc.dram_tensor("bk", [HD], F32, kind="ExternalInput")
    bv = nc.dram_tensor("bv", [HD], F32, kind="ExternalInput")
    woT = nc.dram_tensor("woT", [128, NM, E], BF16, kind="ExternalInput")
    bo2 = nc.dram_tensor("bo2", [E], F32, kind="ExternalInput")
    outT = nc.dram_tensor("outT", [E, S], F32, kind="ExternalOutput")

    with tile.TileContext(nc) as tc:
        mha_tile(
            tc,
            hs[:, :, :, :], wq[:, :, :], wk[:, :, :], wv[:, :, :],
            bq[:], bk[:], bv[:],
            woT[:, :, :], bo2[:], outT[:, :],
        )
    nc.compile()
    return nc


def _pack(x: np.ndarray, nchunk: int) -> np.ndarray:
    """[nchunk*128, cols] row-major -> [128, nchunk, cols] (partition-major)."""
    bf16 = ml_dtypes.bfloat16
    cols = x.shape[1]
    return np.ascontiguousarray(
        x.reshape(nchunk, 128, cols).transpose(1, 0, 2).astype(bf16)
    )


def make_core_inputs(inputs: dict) -> list[dict]:
    """Full inputs -> per-core input maps (core c: batch c//2, head-group c%2)."""
    hsf = np.ascontiguousarray(np.asarray(inputs["hidden_state"], dtype=np.float32))
    Wq = np.asarray(inputs["Wq"], dtype=np.float32)
    Wk = np.asarray(inputs["Wk"], dtype=np.float32)
    Wv = np.asarray(inputs["Wv"], dtype=np.float32)
    Wo = np.asarray(inputs["Wo"], dtype=np.float32)
    bq = np.asarray(inputs["bq"], dtype=np.float32)
    bk = np.asarray(inputs["bk"], dtype=np.float32)
    bv = np.asarray(inputs["bv"], dtype=np.float32)
    bo = np.asarray(inputs["bo"], dtype=np.float32)

    maps = []
    for c in range(8):
        b, g = c // 2, c % 2
        hsl = slice(NH * g, NH * (g + 1))
        fsl = slice(HD * g, HD * (g + 1))
        maps.append(
            {
                "hs": np.ascontiguousarray(
                    hsf[b].reshape(NE, 128, NS, SBW)
                    .transpose(1, 2, 0, 3)
                    .astype(ml_dtypes.bfloat16)
                ),
                "wq": _pack(Wq[hsl].transpose(1, 0, 2).reshape(E, HD), NE),
                "wk": _pack(Wk[hsl].transpose(1, 0, 2).reshape(E, HD), NE),
                "wv": _pack(Wv[hsl].transpose(1, 0, 2).reshape(E, HD), NE),
                "bq": np.ascontiguousarray(bq[hsl].reshape(HD)),
                "bk": np.ascontiguousarray(bk[hsl].reshape(HD)),
                "bv": np.ascontiguousarray(bv[hsl].reshape(HD)),
                "woT": _pack(np.ascontiguousarray(Wo[:, fsl].T), NM),
                "bo2": np.ascontiguousarray(bo / 2.0),
            }
        )
    return maps


def combine_outputs(core_outs: list) -> np.ndarray:
    """Per-core outT partials -> full [B, E, S] output."""
    return np.stack(
        [core_outs[2 * b]["outT"] + core_outs[2 * b + 1]["outT"] for b in range(B)]
    ).astype(np.float32)


from concourse.bass_utils import run_bass_kernel_spmd

N_CORES = 8
_NC_CACHE = None


def _get_nc():
    global _NC_CACHE
    if _NC_CACHE is None:
        _NC_CACHE = build_nc()
    return _NC_CACHE


def kernel(**inputs) -> np.ndarray:
    """Full-input entry point: shard across 8 cores, run, unshard."""
    maps = make_core_inputs(inputs)
    nc = _get_nc()
    res = run_bass_kernel_spmd(nc, maps, core_ids=list(range(N_CORES)))
    outs = res.results
    return np.stack(
        [outs[2 * b]["outT"] + outs[2 * b + 1]["outT"] for b in range(B)]
    ).astype(np.float32)
